# revision 27
# baseline (speedup 1.0000x reference)
"""DeepONet (branch MLP + LoRA-generated per-sample trunk) on 8 TRN2 cores.

Data-parallel over batch: each of the 8 NeuronCores processes 256 samples
(two 128-sample tiles).  v2: all heavy matmuls run as fp8e4m3 DoubleRow
(0.5 cycles/row — 2x bf16) with fp32 PSUM accumulation:

  * branch layers pair REAL contraction chunks ([64,2] split for L0's 128
    sensors, [128,2]-chunk pairs for the 1024-wide layers), so DR wastes
    nothing;
  * trunk Y matmuls (contract dim = 128) zero-pair: lhsT = [hF | 0] and the
    rhs second half is the next 512 A-columns (multiplied by the zero half),
    with A padded by 512 zero cols so the last window stays in bounds;
  * the per-sample k-contraction stays a bf16 diag-matmul chain (fp8 diag
    tiles are costlier to build on DVE than the PE time they would save).

fp8 tensors cross the jax/neuron boundary as uint8 byte views (the compiler
rejects fp8e4m3 I/O dtypes; e4m3 is fine inside the Bass program).
Measured end-to-end rel err vs the fp32 reference is ~2e-3 (tolerance 2e-2).

Branch layer 4 is linear and only feeds the LoRA projection, so V = bw4 @ W1
and W1^T bb4 are folded on the host (exact).  All branch biases are zeros in
setup_inputs(); the host checks this and (only) then skips the bias matmuls
(a nonzero-bias program variant is built otherwise).
"""

import numpy as np
import ml_dtypes

BF = ml_dtypes.bfloat16
F8 = ml_dtypes.float8_e4m3

N_CORES = 8
B = 2048
BL = B // N_CORES          # 256 samples per core
SENSOR = 128
UNITS = 1024
LORA = 64
TU = 128

# trunk param offsets within P=33409
L1W_OFF = 384
L2B_OFF = 16768
L2W_OFF = 16896
L3_OFF = 33280

APAD = 512                 # zero columns appended to A1/A2 for DR windows


# ---------------------------------------------------------------------------
# Walrus here accepts only ONE sync-wait command per instruction; Tile's wait
# assigner attaches several.  Split extras onto standalone EVSEM waits.
# ---------------------------------------------------------------------------
def _install_waitfix():
    import bass_rust as _bass_rust
    import concourse.tile as _tile
    import concourse.mybir as mybir
    from concourse.vector_clock import ScopedClock

    if getattr(_tile.TileContext, "_waitfix_installed", False):
        return

    _MODES = {"sem-ge-imm": "sem-ge", "sem-ge": "sem-ge"}

    def _split(tc, inst):
        si = inst.sync_info
        if si is None or not si.on_wait or len(si.on_wait) <= 1:
            return
        waits = list(si.on_wait)
        keep_idx = 0
        for i, w in enumerate(waits):
            if w.wait_mode not in _MODES or w.wait_reg is not None:
                keep_idx = i
                break
        keep = waits.pop(keep_idx)
        for w in waits:
            assert w.wait_mode in _MODES and w.wait_reg is None
        si.on_wait = [keep]
        inst.sync_info = si
        eng = tc.nc.engines[inst.engine]
        for w in waits:
            sem = _bass_rust.SemaphoreHandle(name=w.ant_name, num=w.id)
            eng.wait_op(sem, int(w.wait_value), _MODES[w.wait_mode])

    _orig_commit = _tile.TileContext._commit_instruction

    def _patched_commit(self, inst, lazy_reg_writes=True):
        si = inst.sync_info
        if (
            si is not None
            and si.on_wait
            and len(si.on_wait) > 1
            and inst.engine != mybir.EngineType.Unassigned
        ):
            cb = self.nc._state.pop_inst_callback()
            try:
                _split(self, inst)
            finally:
                self.nc._state.push_inst_callback(cb)
        return _orig_commit(self, inst, lazy_reg_writes=lazy_reg_writes)

    def _patched_drain(self, tick_clock, wait_clock):
        drain_inst = self.nc.sync.drain()
        wait_clock.add_sem_waits(
            drain_inst.ins, ScopedClock({None: tick_clock.global_clock})
        )
        _split(self, drain_inst.ins)
        self.nc.all_engine_barrier()
        assert self.sems is not None
        popped = self.nc._tile_sem_poison_stack.pop()
        assert popped is self._sem_poison
        self.nc.clear_and_free_semaphores(list(self.sems.allocated().values()))
        self.nc.all_engine_barrier()

    _tile.TileContext._commit_instruction = _patched_commit
    _tile.TileContext._drain_and_barrier = _patched_drain
    _tile.TileContext._waitfix_installed = True


# ---------------------------------------------------------------------------
# Bass program (built once per bias-variant, cached)
# ---------------------------------------------------------------------------
_PROGRAMS = {}


def _build_program(with_bias):
    _install_waitfix()
    from contextlib import ExitStack

    import concourse.bass as bass
    import concourse.mybir as mybir
    from concourse.tile import TileContext

    dt = mybir.dt
    AF = mybir.ActivationFunctionType
    OP = mybir.AluOpType
    DR = mybir.MatmulPerfMode.DoubleRow

    nc = bass.Bass(
        trn_type="TRN2", target_bir_lowering=False, debug=False,
        num_devices=N_CORES,
    )

    # ---- DRAM I/O ----
    uF_d = nc.dram_tensor("uF", [64, 2, BL], dt.float8e4, kind="ExternalInput")
    bw0_d = nc.dram_tensor("bw0", [64, 2, UNITS], dt.float8e4,
                           kind="ExternalInput")
    bw_d = [
        nc.dram_tensor(f"bw{i}", [128, 4, 2, UNITS], dt.float8e4,
                       kind="ExternalInput")
        for i in range(1, 4)
    ]
    V_d = nc.dram_tensor("V", [128, 4, 2, LORA], dt.float8e4,
                         kind="ExternalInput")
    NAW = (LORA * 128 + APAD) // 256          # 34 windows of 256 cols
    A_d = [
        nc.dram_tensor(nm, [128, NAW, 256], dt.float8e4,
                       kind="ExternalInput")
        for nm in ("A1", "A2")
    ]
    # one packed blob for the small trunk tensors: [64, 384+128+129]
    w2s_d = nc.dram_tensor("w2s", [LORA, 641], dt.bfloat16,
                           kind="ExternalInput")
    ut_d = nc.dram_tensor("ut", [128, 4], dt.float32, kind="ExternalInput")
    Dt_d = nc.dram_tensor("Dt", [128, 2, 128], dt.bfloat16,
                          kind="ExternalInput")
    if with_bias:
        bbr_d = nc.dram_tensor("bbr", [1, 5 * UNITS], dt.bfloat16,
                               kind="ExternalInput")
        cb4_d = nc.dram_tensor("cb4", [1, LORA], dt.bfloat16,
                               kind="ExternalInput")
    out_d = nc.dram_tensor("out", [128, 2], dt.float32, kind="ExternalOutput")
    import os
    dbg = os.environ.get("BASSK_DEBUG") == "1"
    if dbg:
        a0_dbg = nc.dram_tensor("a0dbg", [128, 8, BL], dt.float8e4,
                                kind="ExternalOutput")
        a1_dbg = nc.dram_tensor("a1dbg", [128, 8, BL], dt.float8e4,
                                kind="ExternalOutput")
        a2_dbg = nc.dram_tensor("a2dbg", [128, 8, BL], dt.float8e4,
                                kind="ExternalOutput")
        cF_dbg = nc.dram_tensor("cFdbg", [LORA, BL], dt.float32,
                                kind="ExternalOutput")
        a3_dbg = nc.dram_tensor("a3dbg", [128, 8, BL], dt.float8e4,
                                kind="ExternalOutput")
        h1_dbg = nc.dram_tensor("h1dbg", [128, 2, TU], dt.float8e4,
                                kind="ExternalOutput")
        h2_dbg = nc.dram_tensor("h2dbg", [128, 2, TU], dt.float8e4,
                                kind="ExternalOutput")

    with TileContext(nc) as tc, ExitStack() as ctx:
        # ---- SBUF pools ----
        wpool = ctx.enter_context(tc.tile_pool(name="weights", bufs=1))
        apool = ctx.enter_context(tc.tile_pool(name="acts", bufs=2))
        spool = ctx.enter_context(tc.tile_pool(name="small", bufs=1))
        ypool = ctx.enter_context(tc.tile_pool(name="ysb", bufs=3))
        hpool = ctx.enter_context(tc.tile_pool(name="hsb", bufs=4))

        # Junk for PE warmup, identity for diags, ones rows.
        junk = spool.tile([128, 256], dt.bfloat16, name="junk_sb")
        nc.vector.memset(junk[:, :], 0.0)
        iota_i = spool.tile([128, 128], dt.int32, name="iota_sb")
        nc.gpsimd.iota(iota_i[:, :], [[1, 128]], base=0, channel_multiplier=-1)
        I_bf = spool.tile([128, 128], dt.bfloat16, name="ident_sb")
        nc.vector.tensor_scalar(I_bf[:, :], iota_i[:, :], 0, None, OP.is_equal)
        ones_bf = spool.tile([128, 1], dt.bfloat16, name="ones_sb")
        nc.vector.memset(ones_bf[:, :], 1.0)
        if with_bias:
            ones_row = spool.tile([1, BL], dt.bfloat16, name="onesr_sb")
            nc.vector.memset(ones_row[:, :], 1.0)

        # fp8 trunk hF tiles [h | 0]: slot 1 stays zero (DR zero-pair lhsT).
        hFz = {j: spool.tile([128, 2, TU], dt.float8e4, name=f"hFz_{j}")
               for j in range(2)}
        for j in range(2):
            nc.vector.memset(hFz[j][:, 1, :], 0.0)

        # ---- weight loads ----
        # HWDGE streams everything in consumption order.  The packed w2
        # smalls go FIRST (SWDGE descriptor prep on Pool costs ~1.3us +
        # ~45ns/descriptor of Pool-engine time, so only u0/tb ride Pool).
        w2s = spool.tile([LORA, 641], dt.bfloat16, name="w2s_sb")
        nc.sync.dma_start(out=w2s[:, :], in_=w2s_d[:, :])
        uF = wpool.tile([64, 2, BL], dt.float8e4, name="uF_sb")
        nc.sync.dma_start(out=uF[:, :, :], in_=uF_d[:, :, :])
        bw0 = wpool.tile([64, 2, UNITS], dt.float8e4, name="bw0_sb")
        nc.sync.dma_start(out=bw0[:, :, :], in_=bw0_d[:, :, :])
        bws = []
        for i in range(3):
            t = wpool.tile([128, 4, 2, UNITS], dt.float8e4, name=f"bw{i+1}_sb")
            for a in range(4):
                nc.sync.dma_start(out=t[:, a, :, :], in_=bw_d[i][:, a, :, :])
            bws.append(t)
            if i == 0:
                V_sb = spool.tile([128, 4, 2, LORA], dt.float8e4, name="V_sb")
                nc.sync.dma_start(out=V_sb[:, :, :, :], in_=V_d[:, :, :, :])
        A_sb = []
        for i in range(2):
            t = wpool.tile([128, NAW, 256], dt.float8e4, name=f"A{i+1}_sb")
            for lo in range(0, NAW, 8):
                hi = min(lo + 8, NAW)
                nc.sync.dma_start(out=t[:, lo:hi, :], in_=A_d[i][:, lo:hi, :])
            A_sb.append(t)
        # Pool-queue smalls (u0/tb packed into one tiny DMA)
        ut = spool.tile([128, 4], dt.float32, name="ut_sb")
        nc.gpsimd.dma_start(out=ut[:, :], in_=ut_d[:, :])
        Dt_sb = spool.tile([128, 2, 128], dt.bfloat16, name="Dt_sb")
        nc.gpsimd.dma_start(out=Dt_sb[:, :, :], in_=Dt_d[:, :, :])
        if with_bias:
            bbr = spool.tile([1, 5 * UNITS], dt.bfloat16, name="bbr_sb")
            nc.gpsimd.dma_start(out=bbr[:, :], in_=bbr_d[:, :])
            cb4_sb = spool.tile([1, LORA], dt.bfloat16, name="cb4_sb")
            nc.gpsimd.dma_start(out=cb4_sb[:, :], in_=cb4_d[:, :])

        # ---- branch MLP (feature-major, fp8 DoubleRow) ----
        with (
            tc.tile_pool(name="bpsum", bufs=3, space="PSUM") as bpsum,
            tc.tile_pool(name="jpsum", bufs=1, space="PSUM") as jpsum,
        ):
            # PE warmup: the ramp to 2.4 GHz needs ~3us of continuous busy
            # and resets on idle; zero-dependency junk matmuls bridge the
            # initial DMA wait.
            ps_junk = jpsum.tile([128, 256], dt.float32, name="ps_junk",
                                 tag="jps")
            for _ in range(5):
                nc.tensor.matmul(ps_junk[:, :], junk[:, 0:128], junk[:, 0:256])

            def act_pair(l, pss, name):
                """Two [128,1024] tanh (or copy for the c layer) ops -> fp8."""
                nxt = apool.tile([128, 8, BL], dt.float8e4, name=name,
                                 tag="acts")
                for h in range(2):
                    # weights are host-scaled x8 into fp8's normal range;
                    # scale=1/8 undoes it exactly before the tanh
                    nc.scalar.activation(
                        nxt[:, 4 * h:4 * h + 4, :], pss[h][:, :], AF.Tanh,
                        scale=0.125,
                    )
                return nxt

            def branch_layer(l, prev, w_sb):
                """One 1024-wide layer: k-pair-outer DR accumulation."""
                pss = [bpsum.tile([128, 1024], dt.float32,
                                  name=f"ps{l}_{h}", tag="bps")
                       for h in range(2)]
                if with_bias:
                    for h in range(2):
                        for m in range(4 * h, 4 * h + 4):
                            nc.tensor.matmul(
                                pss[h][:, (m % 4) * 256:(m % 4) * 256 + 256],
                                bbr[0:1, l * UNITS + m * 128:
                                    l * UNITS + (m + 1) * 128],
                                ones_row[:, :], start=(m % 2 == 0),
                                stop=False,
                            )
                # PSUM zero regions are bank-granular (2KB = 512 fp32
                # cols): exactly one open accumulation group per bank, so
                # start on the bank's first matmul and stop on its last.
                for a in range(4):
                    rhs = prev[:, 2 * a:2 * a + 2, :]
                    for h in range(2):
                        for m in range(4 * h, 4 * h + 4):
                            nc.tensor.matmul(
                                pss[h][:, (m % 4) * 256:(m % 4) * 256 + 256],
                                w_sb[:, a, :, m * 128:(m + 1) * 128],
                                rhs,
                                start=(a == 0 and m % 2 == 0
                                       and not with_bias),
                                stop=(a == 3 and m % 2 == 1),
                                perf_mode=DR,
                            )
                return act_pair(l, pss, f"act{l}")

            # L0: contract 128 sensors as [64, 2] real pairs.
            ps0 = [bpsum.tile([128, 1024], dt.float32, name=f"ps0_{h}",
                              tag="bps") for h in range(2)]
            if with_bias:
                for h in range(2):
                    for m in range(4 * h, 4 * h + 4):
                        nc.tensor.matmul(
                            ps0[h][:, (m % 4) * 256:(m % 4) * 256 + 256],
                            bbr[0:1, m * 128:(m + 1) * 128],
                            ones_row[:, :], start=(m % 2 == 0), stop=False,
                        )
            for h in range(2):
                for m in range(4 * h, 4 * h + 4):
                    nc.tensor.matmul(
                        ps0[h][:, (m % 4) * 256:(m % 4) * 256 + 256],
                        bw0[:, :, m * 128:(m + 1) * 128],
                        uF[:, :, :],
                        start=(m % 2 == 0 and not with_bias),
                        stop=(m % 2 == 1),
                        perf_mode=DR,
                    )
            act = act_pair(0, ps0, "act0")
            if dbg:
                nc.sync.dma_start(out=a0_dbg[:, :, :], in_=act[:, :, :])
            for l in range(1, 4):
                act = branch_layer(l, act, bws[l - 1])
                if dbg and l == 1:
                    nc.sync.dma_start(out=a1_dbg[:, :, :], in_=act[:, :, :])
                if dbg and l == 2:
                    nc.sync.dma_start(out=a2_dbg[:, :, :], in_=act[:, :, :])

            # ---- c = act3 @ V (+ cb4) in fp8 DR;  c feeds the trunk in bf16.
            ps_cF = bpsum.tile([LORA, BL], dt.float32, name="ps_cF", tag="bps")
            if with_bias:
                nc.tensor.matmul(ps_cF[:, :], cb4_sb[:, :], ones_row[:, :],
                                 start=True, stop=False)
            for a in range(4):
                nc.tensor.matmul(
                    ps_cF[:, :], V_sb[:, a, :, :], act[:, 2 * a:2 * a + 2, :],
                    start=(a == 0 and not with_bias), stop=(a == 3),
                    perf_mode=DR,
                )
            if dbg:
                nc.sync.dma_start(out=a3_dbg[:, :, :], in_=act[:, :, :])
            cF = spool.tile([LORA, BL], dt.bfloat16, name="cF_sb")
            nc.vector.tensor_scalar(cF[:, 0:128], ps_cF[:, 0:128],
                                    0.125, None, OP.mult)
            nc.vector.tensor_scalar(cF[:, 128:256], ps_cF[:, 128:256],
                                    0.125, None, OP.mult)

            if dbg:
                cF32 = spool.tile([LORA, BL], dt.float32, name="cF32_sb")
                nc.vector.tensor_copy(cF32[:, :], cF[:, :])
                nc.sync.dma_start(out=cF_dbg[:, :], in_=cF32[:, :])
            # c batch-major via PE transpose (for diag builds), plus trunk-l0
            # prep (w0sb, Dt) interleaved here exactly like the baseline.
            c_bm = spool.tile([128, 2, LORA], dt.float32, name="cbm_sb")
            w0sbs = []
            for j in range(2):
                tr = bpsum.tile([128, LORA], dt.bfloat16, name=f"trc_{j}",
                                tag="bps")
                nc.tensor.transpose(tr[:, :], cF[:, j * 128:(j + 1) * 128],
                                    I_bf[0:64, 0:64])
                if j == 0:
                    nc.vector.tensor_copy(c_bm[:, j, :], tr[:, :])
                else:
                    nc.scalar.copy(c_bm[:, j, :], tr[:, :])
                ps_l0 = bpsum.tile([128, 128], dt.float32, name=f"psl0_{j}",
                                   tag="bps")
                nc.tensor.matmul(ps_l0[:, :], cF[:, j * 128:(j + 1) * 128],
                                 w2s[:, 128:256])
                w0sb = hpool.tile([128, 128], dt.bfloat16, name=f"w0sb_{j}",
                                  tag="w0sb")
                nc.vector.tensor_copy(w0sb[:, :], ps_l0[:, :])
                w0sbs.append(w0sb)

        # ---- trunk ----
        # Feature-major [feat, batch] throughout; mid layers contract k via
        # the bf16 diag-matmul chain (lhsT = y_sb slice, rhs = diag(c_k)).
        # Y matmuls are fp8 DR zero-pairs: lhsT = [hF | 0], rhs = a sliding
        # 512-col window of A (second 256 cols are junk multiplied by zero).
        D_sb = wpool.tile([128, 2, LORA * 128], dt.bfloat16, name="D_sb")
        out_sb = spool.tile([128, 2], dt.float32, name="out_sb")
        hFs = [None, None]
        # Diag builds: DVE ~94ns, Pool ~280ns each (Pool cannot read PSUM so
        # it does no psy copies — it absorbs the deadline-late diag builds
        # instead).  j0's diags gate the first trunk tile-layer; DVE takes
        # the early ones, Pool the late ones, and j1's mostly go to Pool
        # during the first two tile-layers.
        built_D = {0: set(), 1: set()}

        def emit_D(j, ks, eng):
            for k in ks:
                built_D[j].add(k)
                eng.tensor_scalar(
                    D_sb[:, j, k * 128:(k + 1) * 128], I_bf[:, :],
                    c_bm[:, j, k:k + 1], None, OP.mult,
                )

        with (
            tc.tile_pool(name="ypsum", bufs=3, space="PSUM") as ypsum,
            tc.tile_pool(name="hpsum", bufs=2, space="PSUM") as hpsum,
        ):
            emit_D(0, [k for k in range(40, LORA)], nc.gpsimd)
            emit_D(1, [k for k in range(40, LORA)], nc.gpsimd)

            # trunk layer 0 for both tiles, plus the tail's w3q (it only
            # needs cF, so hoist it off the kernel-end critical path)
            w3qs = []
            for j in range(2):
                cFt = cF[:, j * 128:(j + 1) * 128]
                ps_h1 = hpsum.tile([128, 128], dt.float32, name=f"psh1_{j}",
                                   tag="hps")
                nc.tensor.matmul(ps_h1[:, :], w2s[:, 0:128], cFt,
                                 start=True, stop=False)
                nc.tensor.matmul(ps_h1[:, :], w0sbs[j][:, :], Dt_sb[:, j, :],
                                 start=False, stop=True)
                nc.scalar.activation(hFz[j][:, 0, :], ps_h1[:, :], AF.Tanh)
                hFs[j] = hFz[j]
            fuses = []
            for j in range(2):
                cFt = cF[:, j * 128:(j + 1) * 128]
                psw3 = hpsum.tile([128, 129], dt.float32, name=f"psw3_{j}",
                                  tag="hps")
                nc.tensor.matmul(psw3[:, 1:129], w2s[:, 513:641], cFt,
                                 start=True, stop=False)
                nc.tensor.matmul(psw3[:, 0:1], cFt, w2s[:, 512:513],
                                 start=False, stop=True)
                w3q_sb = hpool.tile([128, 129], dt.bfloat16,
                                    name=f"w3qs_{j}", tag="w3q")
                nc.vector.tensor_copy(w3q_sb[:, :], psw3[:, :])
                w3qs.append(w3q_sb)
                # fuse the trunk-l3 bias with the hard constraint:
                # fuse = b3*t + u0, so the tail is just q*t + fuse
                fuse = hpool.tile([128, 1], dt.float32, name=f"fuse_{j}",
                                  tag="fuse")
                nc.vector.scalar_tensor_tensor(
                    fuse[:, :], w3q_sb[:, 0:1], ut[:, 2 + j:3 + j],
                    ut[:, j:j + 1], OP.mult, OP.add,
                )
                fuses.append(fuse)
                if dbg:
                    nc.sync.dma_start(out=h1_dbg[:, j, :],
                                      in_=hFz[j][:, 0, :])

            def mid_layer(l, j, urgent=(), drip=(), eng_rot=None,
                          tail_fn=None, final=False):
                """One trunk mid layer (l in {0,1} over A_sb[l]) for tile j.

                The diag chain runs two groups behind the Y matmuls so the
                PSUM->SBUF copy of group g overlaps Y[g+1]/diag[g-1] instead
                of stalling the PE.  urgent: (j,k) diag builds emitted on
                DVE up front; drip: (j,k,eng) builds paced across groups.
                """
                cFt = cF[:, j * 128:(j + 1) * 128]
                A = A_sb[l]
                ps_h2 = hpsum.tile([128, 128], dt.float32,
                                   name=f"psh2_{l}_{j}", tag="hps")
                blhs = w2s[:, 256:384] if l == 0 else w2s[:, 384:512]
                nc.tensor.matmul(ps_h2[:, :], blhs, cFt,
                                 start=True, stop=False)
                for bj, k in urgent:
                    emit_D(bj, [k], nc.vector)
                # Drip pacing: the diag chain (2 groups behind) consumes
                # this tile's k = 8(g-2)..8(g-1) at loop iteration g, so all
                # drip builds must be EMITTED (program order!) by iteration
                # 5 — Tile only inserts write->read deps for writes that
                # precede reads in the instruction stream.
                drip = list(drip)
                per_g = (len(drip) + 5) // 6 if drip else 0
                if eng_rot is None:
                    eng_rot = ("act", "dve", "act", "dve", "act", "dve",
                               "act", "dve")

                ysbs = {}

                def emit_diag(g):
                    y_sb = ysbs.pop(g)
                    missing = {g * 8 + kk for kk in range(8)} - built_D[j]
                    assert not missing, (
                        f"diag group {g} of tile {j} reads D ks {missing} "
                        "that have not been emitted yet (program order)")
                    for kk in range(8):
                        k = g * 8 + kk
                        nc.tensor.matmul(
                            ps_h2[:, :],
                            y_sb[:, kk * 128:(kk + 1) * 128],
                            D_sb[:, j, k * 128:(k + 1) * 128],
                            start=False, stop=(k == LORA - 1),
                        )

                for g in range(8):
                    for bj, k, eng in drip[g * per_g:(g + 1) * per_g]:
                        emit_D(bj, [k],
                               nc.vector if eng == "dve" else nc.gpsimd)
                    if g == 2 and tail_fn is not None:
                        tail_fn()
                    psy = ypsum.tile([128, 1024], dt.float32,
                                     name=f"psy{l}_{j}_{g}", tag="yps")
                    for q in range(4):
                        w = g * 4 + q
                        nc.tensor.matmul(
                            psy[:, q * 256:(q + 1) * 256],
                            hFs[j][:, :, :], A[:, w:w + 2, :],
                            start=(q % 2 == 0), stop=(q % 2 == 1),
                            perf_mode=DR,
                        )
                    y_sb = ypool.tile([128, 1024], dt.bfloat16,
                                      name=f"ysb{l}_{j}_{g}", tag="ysb")
                    if final and g >= 6:
                        # last copies gate the kernel tail: split Act+DVE
                        nc.scalar.copy(y_sb[:, 0:512], psy[:, 0:512])
                        nc.vector.tensor_copy(y_sb[:, 512:1024],
                                              psy[:, 512:1024])
                    else:
                        if eng_rot[g] == "act":
                            nc.scalar.copy(y_sb[:, :], psy[:, :])
                        else:
                            nc.vector.tensor_copy(y_sb[:, :], psy[:, :])
                    ysbs[g] = y_sb
                    if g >= 2:
                        emit_diag(g - 2)
                emit_diag(6)
                emit_diag(7)
                if l == 0:
                    if dbg:
                        nc.sync.dma_start(out=h1_dbg[:, j, :],
                                          in_=hFz[j][:, 0, :])
                    nc.scalar.activation(hFz[j][:, 0, :], ps_h2[:, :],
                                         AF.Tanh, scale=0.03125)
                    hFs[j] = hFz[j]
                    if dbg:
                        nc.sync.dma_start(out=h2_dbg[:, j, :],
                                          in_=hFz[j][:, 0, :])
                else:
                    h3 = hpool.tile([128, 128], dt.bfloat16,
                                    name=f"h3_{j}", tag=f"h3{j}")
                    nc.scalar.activation(h3[:, :], ps_h2[:, :], AF.Tanh,
                                         scale=0.03125)
                    hFs[j] = h3

            def tail(j):
                """Trunk layer 3 + hard constraint for batch tile j.

                w3q was precomputed into SBUF at trunk start; the chain here
                is prod (DVE, all-SBUF bf16 = 4x mode), a ones-matmul
                partition reduce, and the final STT.
                """
                prod = hpool.tile([128, 128], dt.bfloat16, name=f"prod_{j}",
                                  tag="prod")
                nc.vector.tensor_tensor(prod[:, :], w3qs[j][:, 1:129],
                                        hFs[j][:, :], OP.mult)
                psq = hpsum.tile([128, 1], dt.float32, name=f"psq_{j}",
                                 tag="hps")
                nc.tensor.matmul(psq[:, :], prod[:, :], ones_bf[:, :])
                nc.vector.scalar_tensor_tensor(
                    out_sb[:, j:j + 1], psq[:, 0:1], ut[:, 2 + j:3 + j],
                    fuses[j][:, :], OP.mult, OP.add,
                )

            # Layer order (0,0),(1,0),(0,1),(1,1): tile j0 runs both its
            # mid layers first so D[j1]'s 64 diag builds get two extra
            # windows before (0,1) consumes them.
            ROT53 = ("act", "act", "dve", "act", "dve", "act", "dve", "act")
            mid_layer(0, 0,
                      urgent=[(0, k) for k in range(16)],
                      drip=[(0, k, "dve") for k in range(16, 40)],
                      eng_rot=ROT53)
            mid_layer(1, 0,
                      drip=[(1, k, "dve") for k in range(0, 40)],
                      eng_rot=ROT53)
            mid_layer(0, 1)
            tail(0)
            mid_layer(1, 1, final=True)
            tail(1)
            nc.sync.dma_start(out=out_d[:, :], in_=out_sb[:, :])

    return nc


def _get_program(with_bias=False):
    if with_bias not in _PROGRAMS:
        _PROGRAMS[with_bias] = _build_program(with_bias)
    return _PROGRAMS[with_bias]


# ---------------------------------------------------------------------------
# host-side prep / gather
# ---------------------------------------------------------------------------
def _f8(x):
    return np.ascontiguousarray(np.asarray(x, np.float32)).astype(F8)


def _bf(x):
    return np.ascontiguousarray(np.asarray(x, np.float32)).astype(BF)


def _u8(x):
    return np.ascontiguousarray(x).view(np.uint8)


def _prep_shared(inputs):
    """Weight-only tensors, identical across cores."""
    W2 = np.asarray(inputs["W2"], np.float32)
    d = {}
    # fp8e4m3 min normal is 2^-6; glorot weights (std ~0.03) and the A
    # blocks (std ~0.008) are largely subnormal there.  Scale them up by
    # exact powers of two (x8 branch/V, x32 A) and fold the inverse into
    # the activation scales on device.
    bw0 = np.asarray(inputs["bw0"], np.float32) * 8.0    # [128, 1024]
    d["bw0"] = _u8(_f8(bw0.reshape(2, 64, UNITS).transpose(1, 0, 2)))
    for i in range(1, 4):
        w = np.asarray(inputs[f"bw{i}"], np.float32) * 8.0
        d[f"bw{i}"] = _u8(_f8(w.reshape(4, 2, 128, UNITS)
                              .transpose(2, 0, 1, 3)))
    W1f = np.asarray(inputs["W1"], np.float64)
    V = (np.asarray(inputs["bw4"], np.float64) @ W1f).astype(np.float32) * 8.0
    d["V"] = _u8(_f8(V.reshape(4, 2, 128, LORA).transpose(2, 0, 1, 3)))
    for nm, off in (("A1", L1W_OFF), ("A2", L2W_OFF)):
        A = W2[:, off:off + 16384].reshape(LORA, 128, 128)
        Aa = np.transpose(A, (1, 0, 2)).reshape(128, LORA * 128)
        Ap = np.zeros((128, LORA * 128 + APAD), np.float32)
        Ap[:, :LORA * 128] = Aa * 32.0
        d[nm] = _u8(_f8(Ap.reshape(128, (LORA * 128 + APAD) // 256, 256)))
    w2s = np.concatenate(
        [W2[:, 0:384], W2[:, L2B_OFF:L2B_OFF + 128],
         W2[:, L3_OFF:L3_OFF + 129]], axis=1).copy()
    w2s[:, 256:512] *= 32.0      # mid-layer bias cols match the A x32 scale
    d["w2s"] = _bf(w2s)

    with_bias = any(np.any(np.asarray(inputs[f"bb{i}"])) for i in range(5))
    if with_bias:
        bbr = np.zeros((1, 5 * UNITS), np.float32)
        for l in range(5):
            bbr[0, l * UNITS:(l + 1) * UNITS] = np.asarray(
                inputs[f"bb{l}"], np.float32)
        d["bbr"] = _bf(bbr * 8.0)
        d["cb4"] = _bf((W1f.T @ np.asarray(inputs["bb4"], np.float64))
                       .astype(np.float32).reshape(1, LORA) * 8.0)
    return d, with_bias


def _prep_core(inputs, core):
    s = slice(core * BL, (core + 1) * BL)
    u = np.asarray(inputs["u"][s], np.float32)           # [256, 128]
    t = np.asarray(inputs["t"][s], np.float32)
    uT = u.T                                             # [128, 256]
    ut = np.concatenate([u[:, 0].reshape(2, 128).T,
                         t.reshape(2, 128).T], axis=1)
    tj = t.reshape(2, 128)
    Dt = np.zeros((128, 2, 128), np.float32)
    for j in range(2):
        np.fill_diagonal(Dt[:, j, :], tj[j])
    return {
        "uF": _u8(_f8(uT.reshape(2, 64, BL).transpose(1, 0, 2))),
        "ut": np.ascontiguousarray(ut),
        "Dt": _bf(Dt),
    }


def _make_in_maps(inputs):
    shared, with_bias = _prep_shared(inputs)
    maps = []
    for core in range(N_CORES):
        d = dict(shared)
        d.update(_prep_core(inputs, core))
        maps.append(d)
    return maps, with_bias


def kernel(**inputs):
    from concourse.bass_utils import run_bass_kernel_spmd

    inputs = {k: np.asarray(v) for k, v in inputs.items()}
    in_maps, with_bias = _make_in_maps(inputs)
    nc = _get_program(with_bias)
    res = None
    last_err = None
    for attempt in range(3):
        try:
            res = run_bass_kernel_spmd(nc, in_maps,
                                       core_ids=list(range(N_CORES)))
            break
        except Exception as e:  # transient NRT/device hiccups recover on retry
            last_err = e
    if res is None:
        raise last_err
    outs = []
    for core in range(N_CORES):
        oc = np.asarray(res.results[core]["out"], np.float32)  # [128, 2]
        outs.append(oc.T.reshape(BL))
    return np.concatenate(outs).astype(np.float32)


# revision 28
# speedup vs baseline: 1.0404x; 1.0404x over previous
"""DeepONet (branch MLP + LoRA-generated per-sample trunk) on 8 TRN2 cores.

Data-parallel over batch: each of the 8 NeuronCores processes 256 samples
(two 128-sample tiles).  v2: all heavy matmuls run as fp8e4m3 DoubleRow
(0.5 cycles/row — 2x bf16) with fp32 PSUM accumulation:

  * branch layers pair REAL contraction chunks ([64,2] split for L0's 128
    sensors, [128,2]-chunk pairs for the 1024-wide layers), so DR wastes
    nothing;
  * trunk Y matmuls (contract dim = 128) zero-pair: lhsT = [hF | 0] and the
    rhs second half is the next 512 A-columns (multiplied by the zero half),
    with A padded by 512 zero cols so the last window stays in bounds;
  * the per-sample k-contraction stays a bf16 diag-matmul chain (fp8 diag
    tiles are costlier to build on DVE than the PE time they would save).

fp8 tensors cross the jax/neuron boundary as uint8 byte views (the compiler
rejects fp8e4m3 I/O dtypes; e4m3 is fine inside the Bass program).
Measured end-to-end rel err vs the fp32 reference is ~2e-3 (tolerance 2e-2).

Branch layer 4 is linear and only feeds the LoRA projection, so V = bw4 @ W1
and W1^T bb4 are folded on the host (exact).  All branch biases are zeros in
setup_inputs(); the host checks this and (only) then skips the bias matmuls
(a nonzero-bias program variant is built otherwise).
"""

import numpy as np
import ml_dtypes

BF = ml_dtypes.bfloat16
F8 = ml_dtypes.float8_e4m3

N_CORES = 8
B = 2048
BL = B // N_CORES          # 256 samples per core
SENSOR = 128
UNITS = 1024
LORA = 64
TU = 128

# trunk param offsets within P=33409
L1W_OFF = 384
L2B_OFF = 16768
L2W_OFF = 16896
L3_OFF = 33280

APAD = 512                 # zero columns appended to A1/A2 for DR windows


# ---------------------------------------------------------------------------
# Walrus here accepts only ONE sync-wait command per instruction; Tile's wait
# assigner attaches several.  Split extras onto standalone EVSEM waits.
# ---------------------------------------------------------------------------
def _install_waitfix():
    import bass_rust as _bass_rust
    import concourse.tile as _tile
    import concourse.mybir as mybir
    from concourse.vector_clock import ScopedClock

    if getattr(_tile.TileContext, "_waitfix_installed", False):
        return

    _MODES = {"sem-ge-imm": "sem-ge", "sem-ge": "sem-ge"}

    def _split(tc, inst):
        si = inst.sync_info
        if si is None or not si.on_wait or len(si.on_wait) <= 1:
            return
        waits = list(si.on_wait)
        keep_idx = 0
        for i, w in enumerate(waits):
            if w.wait_mode not in _MODES or w.wait_reg is not None:
                keep_idx = i
                break
        keep = waits.pop(keep_idx)
        for w in waits:
            assert w.wait_mode in _MODES and w.wait_reg is None
        si.on_wait = [keep]
        inst.sync_info = si
        eng = tc.nc.engines[inst.engine]
        for w in waits:
            sem = _bass_rust.SemaphoreHandle(name=w.ant_name, num=w.id)
            eng.wait_op(sem, int(w.wait_value), _MODES[w.wait_mode])

    _orig_commit = _tile.TileContext._commit_instruction

    def _patched_commit(self, inst, lazy_reg_writes=True):
        si = inst.sync_info
        if (
            si is not None
            and si.on_wait
            and len(si.on_wait) > 1
            and inst.engine != mybir.EngineType.Unassigned
        ):
            cb = self.nc._state.pop_inst_callback()
            try:
                _split(self, inst)
            finally:
                self.nc._state.push_inst_callback(cb)
        return _orig_commit(self, inst, lazy_reg_writes=lazy_reg_writes)

    def _patched_drain(self, tick_clock, wait_clock):
        drain_inst = self.nc.sync.drain()
        wait_clock.add_sem_waits(
            drain_inst.ins, ScopedClock({None: tick_clock.global_clock})
        )
        _split(self, drain_inst.ins)
        self.nc.all_engine_barrier()
        assert self.sems is not None
        popped = self.nc._tile_sem_poison_stack.pop()
        assert popped is self._sem_poison
        self.nc.clear_and_free_semaphores(list(self.sems.allocated().values()))
        self.nc.all_engine_barrier()

    _tile.TileContext._commit_instruction = _patched_commit
    _tile.TileContext._drain_and_barrier = _patched_drain
    _tile.TileContext._waitfix_installed = True


# ---------------------------------------------------------------------------
# Bass program (built once per bias-variant, cached)
# ---------------------------------------------------------------------------
_PROGRAMS = {}


def _build_program(with_bias):
    _install_waitfix()
    from contextlib import ExitStack

    import concourse.bass as bass
    import concourse.mybir as mybir
    from concourse.tile import TileContext

    dt = mybir.dt
    AF = mybir.ActivationFunctionType
    OP = mybir.AluOpType
    DR = mybir.MatmulPerfMode.DoubleRow

    nc = bass.Bass(
        trn_type="TRN2", target_bir_lowering=False, debug=False,
        num_devices=N_CORES,
    )

    # ---- DRAM I/O ----
    # uF and bw0 packed: [64, 2, 256 (uF) + 1024 (bw0)] fp8
    ub_d = nc.dram_tensor("ub", [64, 2, BL + UNITS], dt.float8e4,
                          kind="ExternalInput")
    bw_d = [
        nc.dram_tensor(f"bw{i}", [128, 4, 2, UNITS], dt.float8e4,
                       kind="ExternalInput")
        for i in range(1, 4)
    ]
    V_d = nc.dram_tensor("V", [128, 4, 2, LORA], dt.float8e4,
                         kind="ExternalInput")
    NAW = (LORA * 128 + APAD) // 256          # 34 windows of 256 cols
    A_d = [
        nc.dram_tensor(nm, [128, NAW, 256], dt.float8e4,
                       kind="ExternalInput")
        for nm in ("A1", "A2")
    ]
    # one packed blob for the small trunk tensors: [64, 384+128+129]
    w2s_d = nc.dram_tensor("w2s", [LORA, 641], dt.bfloat16,
                           kind="ExternalInput")
    ut_d = nc.dram_tensor("ut", [128, 4], dt.float32, kind="ExternalInput")
    Dt_d = nc.dram_tensor("Dt", [128, 2, 128], dt.bfloat16,
                          kind="ExternalInput")
    if with_bias:
        bbr_d = nc.dram_tensor("bbr", [1, 5 * UNITS], dt.bfloat16,
                               kind="ExternalInput")
        cb4_d = nc.dram_tensor("cb4", [1, LORA], dt.bfloat16,
                               kind="ExternalInput")
    out_d = nc.dram_tensor("out", [128, 2], dt.float32, kind="ExternalOutput")
    import os
    dbg = os.environ.get("BASSK_DEBUG") == "1"
    if dbg:
        a0_dbg = nc.dram_tensor("a0dbg", [128, 8, BL], dt.float8e4,
                                kind="ExternalOutput")
        a1_dbg = nc.dram_tensor("a1dbg", [128, 8, BL], dt.float8e4,
                                kind="ExternalOutput")
        a2_dbg = nc.dram_tensor("a2dbg", [128, 8, BL], dt.float8e4,
                                kind="ExternalOutput")
        cF_dbg = nc.dram_tensor("cFdbg", [LORA, BL], dt.float32,
                                kind="ExternalOutput")
        a3_dbg = nc.dram_tensor("a3dbg", [128, 8, BL], dt.float8e4,
                                kind="ExternalOutput")
        h1_dbg = nc.dram_tensor("h1dbg", [128, 2, TU], dt.float8e4,
                                kind="ExternalOutput")
        h2_dbg = nc.dram_tensor("h2dbg", [128, 2, TU], dt.float8e4,
                                kind="ExternalOutput")

    with TileContext(nc) as tc, ExitStack() as ctx:
        # ---- SBUF pools ----
        wpool = ctx.enter_context(tc.tile_pool(name="weights", bufs=1))
        apool = ctx.enter_context(tc.tile_pool(name="acts", bufs=2))
        spool = ctx.enter_context(tc.tile_pool(name="small", bufs=1))
        ypool = ctx.enter_context(tc.tile_pool(name="ysb", bufs=4))
        hpool = ctx.enter_context(tc.tile_pool(name="hsb", bufs=4))

        # Junk for PE warmup, identity for diags, ones rows.
        junk = spool.tile([128, 256], dt.bfloat16, name="junk_sb")
        nc.vector.memset(junk[:, :], 0.0)
        iota_i = spool.tile([128, 128], dt.int32, name="iota_sb")
        nc.gpsimd.iota(iota_i[:, :], [[1, 128]], base=0, channel_multiplier=-1)
        I_bf = spool.tile([128, 128], dt.bfloat16, name="ident_sb")
        nc.vector.tensor_scalar(I_bf[:, :], iota_i[:, :], 0, None, OP.is_equal)
        ones_bf = spool.tile([128, 1], dt.bfloat16, name="ones_sb")
        nc.vector.memset(ones_bf[:, :], 1.0)
        if with_bias:
            ones_row = spool.tile([1, BL], dt.bfloat16, name="onesr_sb")
            nc.vector.memset(ones_row[:, :], 1.0)

        # fp8 trunk hF tiles [h | 0]: slot 1 stays zero (DR zero-pair lhsT).
        hFz = {j: spool.tile([128, 2, TU], dt.float8e4, name=f"hFz_{j}")
               for j in range(2)}
        for j in range(2):
            nc.vector.memset(hFz[j][:, 1, :], 0.0)

        # ---- weight loads ----
        # HWDGE streams everything in consumption order; uF+bw0 ride one
        # packed DMA so L0 starts ~1us earlier.  w2s (needed only at the
        # c-phase) goes after the branch weights.
        ub = wpool.tile([64, 2, BL + UNITS], dt.float8e4, name="ub_sb")
        nc.sync.dma_start(out=ub[:, :, :], in_=ub_d[:, :, :])
        bws = []
        for i in range(3):
            t = wpool.tile([128, 4, 2, UNITS], dt.float8e4, name=f"bw{i+1}_sb")
            for a in range(4):
                nc.sync.dma_start(out=t[:, a, :, :], in_=bw_d[i][:, a, :, :])
            bws.append(t)
            if i == 0:
                V_sb = spool.tile([128, 4, 2, LORA], dt.float8e4, name="V_sb")
                nc.sync.dma_start(out=V_sb[:, :, :, :], in_=V_d[:, :, :, :])
            if i == 2:
                w2s = spool.tile([LORA, 641], dt.bfloat16, name="w2s_sb")
                nc.sync.dma_start(out=w2s[:, :], in_=w2s_d[:, :])
        A_sb = []
        for i in range(2):
            t = wpool.tile([128, NAW, 256], dt.float8e4, name=f"A{i+1}_sb")
            for lo in range(0, NAW, 8):
                hi = min(lo + 8, NAW)
                nc.sync.dma_start(out=t[:, lo:hi, :], in_=A_d[i][:, lo:hi, :])
            A_sb.append(t)
        # Pool-queue smalls (u0/tb packed into one tiny DMA)
        ut = spool.tile([128, 4], dt.float32, name="ut_sb")
        nc.gpsimd.dma_start(out=ut[:, :], in_=ut_d[:, :])
        Dt_sb = spool.tile([128, 2, 128], dt.bfloat16, name="Dt_sb")
        nc.gpsimd.dma_start(out=Dt_sb[:, :, :], in_=Dt_d[:, :, :])
        if with_bias:
            bbr = spool.tile([1, 5 * UNITS], dt.bfloat16, name="bbr_sb")
            nc.gpsimd.dma_start(out=bbr[:, :], in_=bbr_d[:, :])
            cb4_sb = spool.tile([1, LORA], dt.bfloat16, name="cb4_sb")
            nc.gpsimd.dma_start(out=cb4_sb[:, :], in_=cb4_d[:, :])

        # ---- branch MLP (feature-major, fp8 DoubleRow) ----
        with (
            tc.tile_pool(name="bpsum", bufs=3, space="PSUM") as bpsum,
            tc.tile_pool(name="jpsum", bufs=1, space="PSUM") as jpsum,
        ):
            # PE warmup: the ramp to 2.4 GHz needs ~3us of continuous busy
            # and resets on idle; zero-dependency junk matmuls bridge the
            # initial DMA wait.
            ps_junk = jpsum.tile([128, 256], dt.float32, name="ps_junk",
                                 tag="jps")
            for _ in range(5):
                nc.tensor.matmul(ps_junk[:, :], junk[:, 0:128], junk[:, 0:256])

            def act_pair(l, pss, name):
                """Four [128,512] tanh ops -> fp8.  Finer grain lets the
                next layer's first DR pair start one act-part earlier.
                scale=1/8 undoes the host x8 weight scaling exactly."""
                nxt = apool.tile([128, 8, BL], dt.float8e4, name=name,
                                 tag="acts")
                for h in range(2):
                    for q in range(2):
                        nc.scalar.activation(
                            nxt[:, 4 * h + 2 * q:4 * h + 2 * q + 2, :],
                            pss[h][:, q * 512:(q + 1) * 512], AF.Tanh,
                            scale=0.125,
                        )
                return nxt

            def branch_layer(l, prev, w_sb):
                """One 1024-wide layer: k-pair-outer DR accumulation."""
                pss = [bpsum.tile([128, 1024], dt.float32,
                                  name=f"ps{l}_{h}", tag="bps")
                       for h in range(2)]
                if with_bias:
                    for h in range(2):
                        for m in range(4 * h, 4 * h + 4):
                            nc.tensor.matmul(
                                pss[h][:, (m % 4) * 256:(m % 4) * 256 + 256],
                                bbr[0:1, l * UNITS + m * 128:
                                    l * UNITS + (m + 1) * 128],
                                ones_row[:, :], start=(m % 2 == 0),
                                stop=False,
                            )
                # PSUM zero regions are bank-granular (2KB = 512 fp32
                # cols): exactly one open accumulation group per bank, so
                # start on the bank's first matmul and stop on its last.
                for a in range(4):
                    rhs = prev[:, 2 * a:2 * a + 2, :]
                    for h in range(2):
                        for m in range(4 * h, 4 * h + 4):
                            nc.tensor.matmul(
                                pss[h][:, (m % 4) * 256:(m % 4) * 256 + 256],
                                w_sb[:, a, :, m * 128:(m + 1) * 128],
                                rhs,
                                start=(a == 0 and m % 2 == 0
                                       and not with_bias),
                                stop=(a == 3 and m % 2 == 1),
                                perf_mode=DR,
                            )
                return act_pair(l, pss, f"act{l}")

            # L0: contract 128 sensors as [64, 2] real pairs.
            ps0 = [bpsum.tile([128, 1024], dt.float32, name=f"ps0_{h}",
                              tag="bps") for h in range(2)]
            if with_bias:
                for h in range(2):
                    for m in range(4 * h, 4 * h + 4):
                        nc.tensor.matmul(
                            ps0[h][:, (m % 4) * 256:(m % 4) * 256 + 256],
                            bbr[0:1, m * 128:(m + 1) * 128],
                            ones_row[:, :], start=(m % 2 == 0), stop=False,
                        )
            for h in range(2):
                for m in range(4 * h, 4 * h + 4):
                    nc.tensor.matmul(
                        ps0[h][:, (m % 4) * 256:(m % 4) * 256 + 256],
                        ub[:, :, BL + m * 128:BL + (m + 1) * 128],
                        ub[:, :, 0:BL],
                        start=(m % 2 == 0 and not with_bias),
                        stop=(m % 2 == 1),
                        perf_mode=DR,
                    )
            act = act_pair(0, ps0, "act0")
            if dbg:
                nc.sync.dma_start(out=a0_dbg[:, :, :], in_=act[:, :, :])
            for l in range(1, 4):
                act = branch_layer(l, act, bws[l - 1])
                if dbg and l == 1:
                    nc.sync.dma_start(out=a1_dbg[:, :, :], in_=act[:, :, :])
                if dbg and l == 2:
                    nc.sync.dma_start(out=a2_dbg[:, :, :], in_=act[:, :, :])

            # ---- c = act3 @ V (+ cb4) in fp8 DR;  c feeds the trunk in bf16.
            ps_cF = bpsum.tile([LORA, BL], dt.float32, name="ps_cF", tag="bps")
            if with_bias:
                nc.tensor.matmul(ps_cF[:, :], cb4_sb[:, :], ones_row[:, :],
                                 start=True, stop=False)
            for a in range(4):
                nc.tensor.matmul(
                    ps_cF[:, :], V_sb[:, a, :, :], act[:, 2 * a:2 * a + 2, :],
                    start=(a == 0 and not with_bias), stop=(a == 3),
                    perf_mode=DR,
                )
            if dbg:
                nc.sync.dma_start(out=a3_dbg[:, :, :], in_=act[:, :, :])
            cF = spool.tile([LORA, BL], dt.bfloat16, name="cF_sb")
            nc.vector.tensor_scalar(cF[:, 0:128], ps_cF[:, 0:128],
                                    0.125, None, OP.mult)
            nc.vector.tensor_scalar(cF[:, 128:256], ps_cF[:, 128:256],
                                    0.125, None, OP.mult)

            if dbg:
                cF32 = spool.tile([LORA, BL], dt.float32, name="cF32_sb")
                nc.vector.tensor_copy(cF32[:, :], cF[:, :])
                nc.sync.dma_start(out=cF_dbg[:, :], in_=cF32[:, :])
            # c batch-major via PE transpose (for diag builds), plus trunk-l0
            # prep (w0sb, Dt) interleaved here exactly like the baseline.
            c_bm = spool.tile([128, 2, LORA], dt.float32, name="cbm_sb")
            w0sbs = []
            for j in range(2):
                tr = bpsum.tile([128, LORA], dt.bfloat16, name=f"trc_{j}",
                                tag="bps")
                nc.tensor.transpose(tr[:, :], cF[:, j * 128:(j + 1) * 128],
                                    I_bf[0:64, 0:64])
                if j == 0:
                    nc.vector.tensor_copy(c_bm[:, j, :], tr[:, :])
                else:
                    nc.scalar.copy(c_bm[:, j, :], tr[:, :])
                ps_l0 = bpsum.tile([128, 128], dt.float32, name=f"psl0_{j}",
                                   tag="bps")
                nc.tensor.matmul(ps_l0[:, :], cF[:, j * 128:(j + 1) * 128],
                                 w2s[:, 128:256])
                w0sb = hpool.tile([128, 128], dt.bfloat16, name=f"w0sb_{j}",
                                  tag="w0sb")
                nc.vector.tensor_copy(w0sb[:, :], ps_l0[:, :])
                w0sbs.append(w0sb)

        # ---- trunk ----
        # Feature-major [feat, batch] throughout; mid layers contract k via
        # the bf16 diag-matmul chain (lhsT = y_sb slice, rhs = diag(c_k)).
        # Y matmuls are fp8 DR zero-pairs: lhsT = [hF | 0], rhs = a sliding
        # 512-col window of A (second 256 cols are junk multiplied by zero).
        D_sb = wpool.tile([128, 2, LORA * 128], dt.bfloat16, name="D_sb")
        out_sb = spool.tile([128, 2], dt.float32, name="out_sb")
        hFs = [None, None]
        # Diag builds: DVE ~94ns, Pool ~280ns each (Pool cannot read PSUM so
        # it does no psy copies — it absorbs the deadline-late diag builds
        # instead).  j0's diags gate the first trunk tile-layer; DVE takes
        # the early ones, Pool the late ones, and j1's mostly go to Pool
        # during the first two tile-layers.
        built_D = {0: set(), 1: set()}

        def emit_D(j, ks, eng):
            for k in ks:
                built_D[j].add(k)
                eng.tensor_scalar(
                    D_sb[:, j, k * 128:(k + 1) * 128], I_bf[:, :],
                    c_bm[:, j, k:k + 1], None, OP.mult,
                )

        with (
            tc.tile_pool(name="ypsum", bufs=3, space="PSUM") as ypsum,
            tc.tile_pool(name="hpsum", bufs=2, space="PSUM") as hpsum,
        ):
            emit_D(0, [k for k in range(40, LORA)], nc.gpsimd)
            emit_D(1, [k for k in range(40, LORA)], nc.gpsimd)

            # trunk layer 0 for both tiles, plus the tail's w3q (it only
            # needs cF, so hoist it off the kernel-end critical path)
            w3qs = []
            for j in range(2):
                cFt = cF[:, j * 128:(j + 1) * 128]
                ps_h1 = hpsum.tile([128, 128], dt.float32, name=f"psh1_{j}",
                                   tag="hps")
                nc.tensor.matmul(ps_h1[:, :], w2s[:, 0:128], cFt,
                                 start=True, stop=False)
                nc.tensor.matmul(ps_h1[:, :], w0sbs[j][:, :], Dt_sb[:, j, :],
                                 start=False, stop=True)
                nc.scalar.activation(hFz[j][:, 0, :], ps_h1[:, :], AF.Tanh)
                hFs[j] = hFz[j]
            fuses = []
            for j in range(2):
                cFt = cF[:, j * 128:(j + 1) * 128]
                psw3 = hpsum.tile([128, 129], dt.float32, name=f"psw3_{j}",
                                  tag="hps")
                nc.tensor.matmul(psw3[:, 1:129], w2s[:, 513:641], cFt,
                                 start=True, stop=False)
                nc.tensor.matmul(psw3[:, 0:1], cFt, w2s[:, 512:513],
                                 start=False, stop=True)
                w3q_sb = hpool.tile([128, 129], dt.bfloat16,
                                    name=f"w3qs_{j}", tag="w3q")
                nc.vector.tensor_copy(w3q_sb[:, :], psw3[:, :])
                w3qs.append(w3q_sb)
                # fuse the trunk-l3 bias with the hard constraint:
                # fuse = b3*t + u0, so the tail is just q*t + fuse
                fuse = hpool.tile([128, 1], dt.float32, name=f"fuse_{j}",
                                  tag="fuse")
                nc.vector.scalar_tensor_tensor(
                    fuse[:, :], w3q_sb[:, 0:1], ut[:, 2 + j:3 + j],
                    ut[:, j:j + 1], OP.mult, OP.add,
                )
                fuses.append(fuse)
                if dbg:
                    nc.sync.dma_start(out=h1_dbg[:, j, :],
                                      in_=hFz[j][:, 0, :])

            def mid_layer(l, j, urgent=(), drip=(), eng_rot=None,
                          tail_fn=None, final=False):
                """One trunk mid layer (l in {0,1} over A_sb[l]) for tile j.

                The diag chain runs three groups behind the Y matmuls so the
                PSUM->SBUF copy of group g overlaps Y[g+1]/diag[g-1] instead
                of stalling the PE.  urgent: (j,k) diag builds emitted on
                DVE up front; drip: (j,k,eng) builds paced across groups.
                """
                cFt = cF[:, j * 128:(j + 1) * 128]
                A = A_sb[l]
                ps_h2 = hpsum.tile([128, 128], dt.float32,
                                   name=f"psh2_{l}_{j}", tag="hps")
                blhs = w2s[:, 256:384] if l == 0 else w2s[:, 384:512]
                nc.tensor.matmul(ps_h2[:, :], blhs, cFt,
                                 start=True, stop=False)
                for bj, k in urgent:
                    emit_D(bj, [k], nc.vector)
                # Drip pacing: the diag chain (2 groups behind) consumes
                # this tile's k = 8(g-2)..8(g-1) at loop iteration g, so all
                # drip builds must be EMITTED (program order!) by iteration
                # 5 — Tile only inserts write->read deps for writes that
                # precede reads in the instruction stream.
                drip = list(drip)
                per_g = (len(drip) + 5) // 6 if drip else 0
                if eng_rot is None:
                    eng_rot = ("act", "dve", "act", "dve", "act", "dve",
                               "act", "dve")

                ysbs = {}

                def emit_diag(g):
                    y_sb = ysbs.pop(g)
                    missing = {g * 8 + kk for kk in range(8)} - built_D[j]
                    assert not missing, (
                        f"diag group {g} of tile {j} reads D ks {missing} "
                        "that have not been emitted yet (program order)")
                    for kk in range(8):
                        k = g * 8 + kk
                        nc.tensor.matmul(
                            ps_h2[:, :],
                            y_sb[:, kk * 128:(kk + 1) * 128],
                            D_sb[:, j, k * 128:(k + 1) * 128],
                            start=False, stop=(k == LORA - 1),
                        )

                for g in range(8):
                    for bj, k, eng in drip[g * per_g:(g + 1) * per_g]:
                        emit_D(bj, [k],
                               nc.vector if eng == "dve" else nc.gpsimd)
                    if g == 2 and tail_fn is not None:
                        tail_fn()
                    psy = ypsum.tile([128, 1024], dt.float32,
                                     name=f"psy{l}_{j}_{g}", tag="yps")
                    for q in range(4):
                        w = g * 4 + q
                        nc.tensor.matmul(
                            psy[:, q * 256:(q + 1) * 256],
                            hFs[j][:, :, :], A[:, w:w + 2, :],
                            start=(q % 2 == 0), stop=(q % 2 == 1),
                            perf_mode=DR,
                        )
                    y_sb = ypool.tile([128, 1024], dt.bfloat16,
                                      name=f"ysb{l}_{j}_{g}", tag="ysb")
                    if final and g >= 6:
                        # last copies gate the kernel tail: split Act+DVE
                        nc.scalar.copy(y_sb[:, 0:512], psy[:, 0:512])
                        nc.vector.tensor_copy(y_sb[:, 512:1024],
                                              psy[:, 512:1024])
                    else:
                        if eng_rot[g] == "act":
                            nc.scalar.copy(y_sb[:, :], psy[:, :])
                        else:
                            nc.vector.tensor_copy(y_sb[:, :], psy[:, :])
                    ysbs[g] = y_sb
                    if g >= 3:
                        emit_diag(g - 3)
                emit_diag(5)
                emit_diag(6)
                emit_diag(7)
                if l == 0:
                    if dbg:
                        nc.sync.dma_start(out=h1_dbg[:, j, :],
                                          in_=hFz[j][:, 0, :])
                    nc.scalar.activation(hFz[j][:, 0, :], ps_h2[:, :],
                                         AF.Tanh, scale=0.03125)
                    hFs[j] = hFz[j]
                    if dbg:
                        nc.sync.dma_start(out=h2_dbg[:, j, :],
                                          in_=hFz[j][:, 0, :])
                else:
                    h3 = hpool.tile([128, 128], dt.bfloat16,
                                    name=f"h3_{j}", tag=f"h3{j}")
                    nc.scalar.activation(h3[:, :], ps_h2[:, :], AF.Tanh,
                                         scale=0.03125)
                    hFs[j] = h3

            def tail(j):
                """Trunk layer 3 + hard constraint for batch tile j.

                w3q was precomputed into SBUF at trunk start; the chain here
                is prod (DVE, all-SBUF bf16 = 4x mode), a ones-matmul
                partition reduce, and the final STT.
                """
                prod = hpool.tile([128, 128], dt.bfloat16, name=f"prod_{j}",
                                  tag="prod")
                nc.vector.tensor_tensor(prod[:, :], w3qs[j][:, 1:129],
                                        hFs[j][:, :], OP.mult)
                psq = hpsum.tile([128, 1], dt.float32, name=f"psq_{j}",
                                 tag="hps")
                nc.tensor.matmul(psq[:, :], prod[:, :], ones_bf[:, :])
                nc.vector.scalar_tensor_tensor(
                    out_sb[:, j:j + 1], psq[:, 0:1], ut[:, 2 + j:3 + j],
                    fuses[j][:, :], OP.mult, OP.add,
                )

            # Layer order (0,0),(1,0),(0,1),(1,1): tile j0 runs both its
            # mid layers first so D[j1]'s 64 diag builds get two extra
            # windows before (0,1) consumes them.
            ROT53 = ("act", "act", "dve", "act", "dve", "act", "dve", "act")
            mid_layer(0, 0,
                      urgent=[(0, k) for k in range(16)],
                      drip=[(0, k, "dve") for k in range(16, 40)],
                      eng_rot=ROT53)
            mid_layer(1, 0,
                      drip=[(1, k, "dve") for k in range(0, 40)],
                      eng_rot=ROT53)
            mid_layer(0, 1)
            tail(0)
            mid_layer(1, 1, final=True)
            tail(1)
            nc.sync.dma_start(out=out_d[:, :], in_=out_sb[:, :])

    return nc


def _get_program(with_bias=False):
    if with_bias not in _PROGRAMS:
        _PROGRAMS[with_bias] = _build_program(with_bias)
    return _PROGRAMS[with_bias]


# ---------------------------------------------------------------------------
# host-side prep / gather
# ---------------------------------------------------------------------------
def _f8(x):
    return np.ascontiguousarray(np.asarray(x, np.float32)).astype(F8)


def _bf(x):
    return np.ascontiguousarray(np.asarray(x, np.float32)).astype(BF)


def _u8(x):
    return np.ascontiguousarray(x).view(np.uint8)


def _prep_shared(inputs):
    """Weight-only tensors, identical across cores."""
    W2 = np.asarray(inputs["W2"], np.float32)
    d = {}
    # fp8e4m3 min normal is 2^-6; glorot weights (std ~0.03) and the A
    # blocks (std ~0.008) are largely subnormal there.  Scale them up by
    # exact powers of two (x8 branch/V, x32 A) and fold the inverse into
    # the activation scales on device.
    bw0 = np.asarray(inputs["bw0"], np.float32) * 8.0    # [128, 1024]
    d["_bw0dr"] = _f8(bw0.reshape(2, 64, UNITS).transpose(1, 0, 2))
    for i in range(1, 4):
        w = np.asarray(inputs[f"bw{i}"], np.float32) * 8.0
        d[f"bw{i}"] = _u8(_f8(w.reshape(4, 2, 128, UNITS)
                              .transpose(2, 0, 1, 3)))
    W1f = np.asarray(inputs["W1"], np.float64)
    V = (np.asarray(inputs["bw4"], np.float64) @ W1f).astype(np.float32) * 8.0
    d["V"] = _u8(_f8(V.reshape(4, 2, 128, LORA).transpose(2, 0, 1, 3)))
    for nm, off in (("A1", L1W_OFF), ("A2", L2W_OFF)):
        A = W2[:, off:off + 16384].reshape(LORA, 128, 128)
        Aa = np.transpose(A, (1, 0, 2)).reshape(128, LORA * 128)
        Ap = np.zeros((128, LORA * 128 + APAD), np.float32)
        Ap[:, :LORA * 128] = Aa * 32.0
        d[nm] = _u8(_f8(Ap.reshape(128, (LORA * 128 + APAD) // 256, 256)))
    w2s = np.concatenate(
        [W2[:, 0:384], W2[:, L2B_OFF:L2B_OFF + 128],
         W2[:, L3_OFF:L3_OFF + 129]], axis=1).copy()
    w2s[:, 256:512] *= 32.0      # mid-layer bias cols match the A x32 scale
    d["w2s"] = _bf(w2s)

    with_bias = any(np.any(np.asarray(inputs[f"bb{i}"])) for i in range(5))
    if with_bias:
        bbr = np.zeros((1, 5 * UNITS), np.float32)
        for l in range(5):
            bbr[0, l * UNITS:(l + 1) * UNITS] = np.asarray(
                inputs[f"bb{l}"], np.float32)
        d["bbr"] = _bf(bbr * 8.0)
        d["cb4"] = _bf((W1f.T @ np.asarray(inputs["bb4"], np.float64))
                       .astype(np.float32).reshape(1, LORA) * 8.0)
    return d, with_bias


def _prep_core(inputs, core):
    s = slice(core * BL, (core + 1) * BL)
    u = np.asarray(inputs["u"][s], np.float32)           # [256, 128]
    t = np.asarray(inputs["t"][s], np.float32)
    uT = u.T                                             # [128, 256]
    ut = np.concatenate([u[:, 0].reshape(2, 128).T,
                         t.reshape(2, 128).T], axis=1)
    tj = t.reshape(2, 128)
    Dt = np.zeros((128, 2, 128), np.float32)
    for j in range(2):
        np.fill_diagonal(Dt[:, j, :], tj[j])
    return {
        "_uFdr": _f8(uT.reshape(2, 64, BL).transpose(1, 0, 2)),
        "ut": np.ascontiguousarray(ut),
        "Dt": _bf(Dt),
    }


def _make_in_maps(inputs):
    shared, with_bias = _prep_shared(inputs)
    maps = []
    for core in range(N_CORES):
        d = dict(shared)
        d.update(_prep_core(inputs, core))
        ub = np.concatenate([d.pop("_uFdr"), d["_bw0dr"]], axis=2)
        d.pop("_bw0dr")
        d["ub"] = _u8(ub)
        maps.append(d)
    return maps, with_bias


def kernel(**inputs):
    from concourse.bass_utils import run_bass_kernel_spmd

    inputs = {k: np.asarray(v) for k, v in inputs.items()}
    in_maps, with_bias = _make_in_maps(inputs)
    nc = _get_program(with_bias)
    res = None
    last_err = None
    for attempt in range(3):
        try:
            res = run_bass_kernel_spmd(nc, in_maps,
                                       core_ids=list(range(N_CORES)))
            break
        except Exception as e:  # transient NRT/device hiccups recover on retry
            last_err = e
    if res is None:
        raise last_err
    outs = []
    for core in range(N_CORES):
        oc = np.asarray(res.results[core]["out"], np.float32)  # [128, 2]
        outs.append(oc.T.reshape(BL))
    return np.concatenate(outs).astype(np.float32)
